# revision 6
# baseline (speedup 1.0000x reference)
"""Segment-mean (MeanAggregator) Trainium2 kernel.

Sorted segment ids -> shard *segments* evenly across the 8 cores (host
snaps edge ranges to segment boundaries) so each core owns a disjoint row
range of the output: no cross-core reduction.

Per core, segments are processed in uniform windows of 128 segments held
as a [128 segs, 65] PSUM accumulator (64 sums + count).  Edges stream in
chunks of 128 (the matmul contraction dim).  For each window, ONE
vector-engine tensor_tensor (with stride-0 broadcast APs) builds the
one-hot block O[edge, chunk, seg] = (loc[edge,chunk] == seg), and each
chunk does TensorE matmul(s) psum += O_c.T @ [values | ones].  Padding
slots carry loc = -1 whose one-hot row is all zero.

MODE (dtype of the matmul operands):
  - "split": values split into bf16 hi + bf16 lo, both accumulated into
    the same fp32 PSUM (hi's ones-col carries the count, lo's is 0).
    Full-rate PE with FWL weight loads; ~1e-6 relative error.
  - "f16":   values cast to fp16 (halves the input DMA); ~3e-4 rel err.
  - "f32":   exact fp32 (PE runs 2 half-rate passes per matmul; slow).

The per-window chunk count n_c[w] is the max over cores, so the program
is identical on all 8 cores (SPMD) while per-core data differs.
"""

import math
import os

import ml_dtypes
import numpy as np

import concourse.bacc as bacc
import concourse.mybir as mybir
import concourse.tile as tile
from concourse.bass_utils import run_bass_kernel_spmd

N_CORES = 8
P = 128       # edges per chunk == contraction dim
W_SEGS = int(os.environ.get("KWSEGS", "64"))  # segments per PSUM window
D = 64

MODE = os.environ.get("KMODE", "split")

_PROGRAM_CACHE = {}


def _build_program(mode, n_cs, vals_offs, locs_offs):
    key = (mode, tuple(n_cs))
    if key in _PROGRAM_CACHE:
        return _PROGRAM_CACHE[key]
    f32 = mybir.dt.float32
    mmdt = {
        "split": mybir.dt.bfloat16,
        "f16": mybir.dt.float16,
        "f32": f32,
    }[mode]
    n_pass = 2 if mode == "split" else 1
    n_win = len(n_cs)
    nc = bacc.Bacc(
        "TRN2", target_bir_lowering=False, debug=False, num_devices=N_CORES
    )
    vals = nc.dram_tensor("vals", [vals_offs[-1]], mmdt, kind="ExternalInput")
    locs = nc.dram_tensor("locs", [locs_offs[-1]], f32, kind="ExternalInput")
    outp = nc.dram_tensor("outp", [n_win * W_SEGS, D], f32, kind="ExternalOutput")

    row = n_pass * (D + 1)  # columns per chunk in the vals tile

    with tile.TileContext(nc) as tc:
        with (
            tc.tile_pool(name="const", bufs=1) as cpool,
            tc.tile_pool(name="vals", bufs=3) as vpool,
            tc.tile_pool(name="locs", bufs=3) as lpool,
            tc.tile_pool(name="oh", bufs=3) as opool,
            tc.tile_pool(name="acc", bufs=2, space="PSUM") as ppool,
            tc.tile_pool(name="res", bufs=3) as rpool,
        ):
            iota = cpool.tile([P, W_SEGS], f32)
            nc.gpsimd.iota(
                iota[:],
                pattern=[[1, W_SEGS]],
                base=0,
                channel_multiplier=0,
                allow_small_or_imprecise_dtypes=True,
            )
            max_nc = max(n_cs)
            for w in range(n_win):
                n_c = n_cs[w]
                vt = vpool.tile([P, max_nc * row], mmdt, tag="vt")
                nc.sync.dma_start(
                    out=vt[:, : n_c * row],
                    in_=vals[vals_offs[w] : vals_offs[w + 1]].rearrange(
                        "(p f) -> p f", p=P
                    ),
                )
                vt4 = vt[:].rearrange("p (c s e) -> p c s e", s=n_pass, e=D + 1)
                lt = lpool.tile([P, max_nc], f32, tag="lt")
                nc.sync.dma_start(
                    out=lt[:, :n_c],
                    in_=locs[locs_offs[w] : locs_offs[w + 1]].rearrange(
                        "(p f) -> p f", p=P
                    ),
                )
                oh = opool.tile([P, max_nc * W_SEGS], mmdt, tag="oh")
                oh3 = oh[:].rearrange("p (c j) -> p c j", j=W_SEGS)
                nc.vector.tensor_tensor(
                    out=oh3[:, :n_c, :],
                    in0=lt[:, :n_c, None].to_broadcast([P, n_c, W_SEGS]),
                    in1=iota[:, None, :].to_broadcast([P, n_c, W_SEGS]),
                    op=mybir.AluOpType.is_equal,
                )
                ps = ppool.tile([W_SEGS, D + 1], f32)
                for c in range(n_c):
                    for s in range(n_pass):
                        nc.tensor.matmul(
                            out=ps[:],
                            lhsT=oh3[:, c, :],
                            rhs=vt4[:, c, s, :],
                            start=(c == 0 and s == 0),
                            stop=(c == n_c - 1 and s == n_pass - 1),
                        )
                cnt = rpool.tile([W_SEGS, 1], f32, tag="cnt")
                nc.vector.tensor_scalar(
                    out=cnt[:],
                    in0=ps[:, D : D + 1],
                    scalar1=1.0,
                    scalar2=None,
                    op0=mybir.AluOpType.max,
                )
                rec = rpool.tile([W_SEGS, 1], f32, tag="rec")
                nc.vector.reciprocal(out=rec[:], in_=cnt[:])
                ot = rpool.tile([W_SEGS, D], f32, tag="ot")
                nc.scalar.mul(out=ot[:], in_=ps[:, 0:D], mul=rec[:, 0:1])
                nc.sync.dma_start(
                    out=outp[w * W_SEGS : (w + 1) * W_SEGS, :], in_=ot[:]
                )
    nc.compile()
    _PROGRAM_CACHE[key] = nc
    return nc


def _prepare_inputs(values, segment_ids, num_segments, mode=MODE):
    E, d = values.shape
    assert d == D
    N = int(num_segments)
    sids = np.ascontiguousarray(np.asarray(segment_ids, dtype=np.int64))
    vals_f = np.ascontiguousarray(np.asarray(values), dtype=np.float32)

    n_pass = 2 if mode == "split" else 1
    np_mmdt = {
        "split": ml_dtypes.bfloat16,
        "f16": np.float16,
        "f32": np.float32,
    }[mode]
    row = n_pass * (D + 1)

    segs_per_core = math.ceil(N / N_CORES)
    n_win = math.ceil(segs_per_core / W_SEGS)

    bnds = []
    for k in range(N_CORES):
        for w in range(n_win):
            lo = min(N, k * segs_per_core + w * W_SEGS)
            hi = min(N, k * segs_per_core + min((w + 1) * W_SEGS, segs_per_core))
            bnds.append((lo, hi))
    seg_bnds = np.asarray(bnds, dtype=np.int64)
    e_lo = np.searchsorted(sids, seg_bnds[:, 0], side="left")
    e_hi = np.searchsorted(sids, seg_bnds[:, 1], side="left")
    n_edges = (e_hi - e_lo).reshape(N_CORES, n_win)
    n_cs = np.maximum(1, (n_edges.max(axis=0) + P - 1) // P).astype(int)

    vals_offs = np.concatenate([[0], np.cumsum(n_cs * P * row)]).astype(int)
    locs_offs = np.concatenate([[0], np.cumsum(n_cs * P)]).astype(int)

    in_maps = []
    for k in range(N_CORES):
        vals_k = np.zeros(vals_offs[-1], dtype=np_mmdt)
        locs_k = np.full(locs_offs[-1], -1.0, dtype=np.float32)
        for w in range(n_win):
            i = k * n_win + w
            lo, hi = e_lo[i], e_hi[i]
            ne = int(hi - lo)
            n_c = int(n_cs[w])
            slots = n_c * P
            blk = np.zeros((slots, n_pass, D + 1), dtype=np.float32)
            blk[:, 0, D] = 1.0
            if ne > 0:
                v = vals_f[lo:hi]
                if mode == "split":
                    hi16 = v.astype(ml_dtypes.bfloat16)
                    blk[:ne, 0, :D] = hi16
                    blk[:ne, 1, :D] = v - hi16.astype(np.float32)
                else:
                    blk[:ne, 0, :D] = v
            blk = blk.reshape(n_c, P, row).transpose(1, 0, 2)
            vals_k[vals_offs[w] : vals_offs[w + 1]] = (
                blk.astype(np_mmdt).reshape(-1)
            )
            lblk = np.full(slots, -1.0, dtype=np.float32)
            if ne > 0:
                lblk[:ne] = (sids[lo:hi] - seg_bnds[i, 0]).astype(np.float32)
            locs_k[locs_offs[w] : locs_offs[w + 1]] = (
                lblk.reshape(n_c, P).transpose(1, 0).reshape(-1)
            )
        in_maps.append({"vals": vals_k, "locs": locs_k})
    return in_maps, list(n_cs), list(vals_offs), list(locs_offs), segs_per_core, N


def kernel(values, segment_ids, num_segments):
    mode = MODE
    in_maps, n_cs, vals_offs, locs_offs, segs_per_core, N = _prepare_inputs(
        values, segment_ids, num_segments, mode
    )
    nc = _build_program(mode, n_cs, vals_offs, locs_offs)
    trace = bool(int(os.environ.get("KTRACE", "0")))
    res = run_bass_kernel_spmd(
        nc,
        in_maps,
        list(range(N_CORES)),
        trace=trace,
        tmpdir=os.environ.get("KTRACE_DIR") or None,
    )
    global LAST_RESULT
    LAST_RESULT = res
    parts = []
    for k in range(N_CORES):
        take = min(segs_per_core, N - k * segs_per_core)
        if take > 0:
            parts.append(res.results[k]["outp"][:take])
    return np.concatenate(parts, axis=0).astype(np.float32)


# revision 10
# speedup vs baseline: 1.5483x; 1.5483x over previous
"""Segment-mean (MeanAggregator) Trainium2 kernel.

Sorted segment ids -> shard *segments* evenly across the 8 cores (host
snaps edge ranges to segment boundaries) so each core owns a disjoint row
range of the output: no cross-core reduction.

Per core, segments are processed in quads of 4 sub-windows x 32 segments
(= 128 output rows) sharing one [128, 65] fp32 PSUM accumulator: sub s
owns partitions [32s, 32s+32) via matmul tile_position=(0, 32s) col-tiling.
Edges stream in chunks of 128 (the contraction dim).  Per quad, ONE
vector-engine tensor_tensor (stride-0 broadcast APs) builds the narrow
one-hot block O[edge, chunk, j] = (loc[edge,chunk] == j), j in [0,32), and
each chunk runs TensorE matmul(s) ps[32s:32s+32] += O_c.T @ [vals | ones].
Padding slots carry loc = -1 whose one-hot row is all zero.

MODE: "split" = values as bf16 hi + bf16 lo into the same fp32 PSUM
(full-rate PE, ~2e-6 rel err); "f16" = fp16 values (~2e-4); "f32" exact
but PE runs 2 half-rate passes per matmul.

Chunk counts per (quad, sub) are maxed over cores, so the SPMD program is
identical on all 8 cores while per-core data differs.
"""

import math
import os

import ml_dtypes
import numpy as np

import concourse.bacc as bacc
import concourse.mybir as mybir
import concourse.tile as tile
from concourse.bass_utils import run_bass_kernel_spmd

N_CORES = 8
P = 128   # edges per chunk == contraction dim
SUB = 32  # segments per sub-window (one-hot width, PSUM partition slice)
SPQ = 4   # sub-windows per quad
QUAD = SUB * SPQ  # output rows per quad
D = 64

MODE = os.environ.get("KMODE", "split")

_PROGRAM_CACHE = {}


def _build_program(mode, n_cs, vals_offs, locs_offs):
    """n_cs: [n_quad][SPQ] chunk counts; *_offs: flat offsets per quad."""
    key = (mode, tuple(tuple(q) for q in n_cs))
    if key in _PROGRAM_CACHE:
        return _PROGRAM_CACHE[key]
    f32 = mybir.dt.float32
    mmdt = {
        "split": mybir.dt.bfloat16,
        "f16": mybir.dt.float16,
        "f32": f32,
    }[mode]
    n_pass = 2 if mode == "split" else 1
    n_quad = len(n_cs)
    row = n_pass * (D + 1)
    nc = bacc.Bacc(
        "TRN2", target_bir_lowering=False, debug=False, num_devices=N_CORES
    )
    vals = nc.dram_tensor("vals", [vals_offs[-1]], mmdt, kind="ExternalInput")
    locs = nc.dram_tensor("locs", [locs_offs[-1]], f32, kind="ExternalInput")
    outp = nc.dram_tensor("outp", [n_quad * QUAD, D], f32, kind="ExternalOutput")

    with tile.TileContext(nc) as tc:
        with (
            tc.tile_pool(name="const", bufs=1) as cpool,
            tc.tile_pool(name="vals", bufs=3) as vpool,
            tc.tile_pool(name="locs", bufs=3) as lpool,
            tc.tile_pool(name="oh", bufs=3) as opool,
            tc.tile_pool(name="acc", bufs=2, space="PSUM") as ppool,
            tc.tile_pool(name="res", bufs=3) as rpool,
        ):
            iota = cpool.tile([P, SUB], f32)
            nc.gpsimd.iota(
                iota[:],
                pattern=[[1, SUB]],
                base=0,
                channel_multiplier=0,
                allow_small_or_imprecise_dtypes=True,
            )
            max_ncq = max(sum(q) for q in n_cs)
            for w in range(n_quad):
                n_c_q = sum(n_cs[w])
                vt = vpool.tile([P, max_ncq * row], mmdt, tag="vt")
                nc.sync.dma_start(
                    out=vt[:, : n_c_q * row],
                    in_=vals[vals_offs[w] : vals_offs[w + 1]].rearrange(
                        "(p f) -> p f", p=P
                    ),
                )
                vt4 = vt[:].rearrange("p (c s e) -> p c s e", s=n_pass, e=D + 1)
                lt = lpool.tile([P, max_ncq], f32, tag="lt")
                nc.sync.dma_start(
                    out=lt[:, :n_c_q],
                    in_=locs[locs_offs[w] : locs_offs[w + 1]].rearrange(
                        "(p f) -> p f", p=P
                    ),
                )
                oh = opool.tile([P, max_ncq * SUB], mmdt, tag="oh")
                oh3 = oh[:].rearrange("p (c j) -> p c j", j=SUB)
                nc.vector.tensor_tensor(
                    out=oh3[:, :n_c_q, :],
                    in0=lt[:, :n_c_q, None].to_broadcast([P, n_c_q, SUB]),
                    in1=iota[:, None, :].to_broadcast([P, n_c_q, SUB]),
                    op=mybir.AluOpType.is_equal,
                )
                ps = ppool.tile([QUAD, D + 1], f32)
                cg = 0
                for sub in range(SPQ):
                    ncs_ = n_cs[w][sub]
                    for c in range(ncs_):
                        for s in range(n_pass):
                            nc.tensor.matmul(
                                out=ps[SUB * sub : SUB * (sub + 1), :],
                                lhsT=oh3[:, cg, :],
                                rhs=vt4[:, cg, s, :],
                                start=(c == 0 and s == 0),
                                stop=(c == ncs_ - 1 and s == n_pass - 1),
                                tile_position=(0, SUB * sub),
                            )
                        cg += 1
                cnt = rpool.tile([QUAD, 1], f32, tag="cnt")
                nc.vector.tensor_scalar(
                    out=cnt[:],
                    in0=ps[:, D : D + 1],
                    scalar1=1.0,
                    scalar2=None,
                    op0=mybir.AluOpType.max,
                )
                rec = rpool.tile([QUAD, 1], f32, tag="rec")
                nc.vector.reciprocal(out=rec[:], in_=cnt[:])
                ot = rpool.tile([QUAD, D], f32, tag="ot")
                nc.scalar.mul(out=ot[:], in_=ps[:, 0:D], mul=rec[:, 0:1])
                nc.sync.dma_start(
                    out=outp[w * QUAD : (w + 1) * QUAD, :], in_=ot[:]
                )
    nc.compile()
    _PROGRAM_CACHE[key] = nc
    return nc


def _prepare_inputs(values, segment_ids, num_segments, mode=MODE):
    E, d = values.shape
    assert d == D
    N = int(num_segments)
    sids = np.ascontiguousarray(np.asarray(segment_ids, dtype=np.int64))
    vals_f = np.ascontiguousarray(np.asarray(values), dtype=np.float32)

    n_pass = 2 if mode == "split" else 1
    np_mmdt = {
        "split": ml_dtypes.bfloat16,
        "f16": np.float16,
        "f32": np.float32,
    }[mode]
    row = n_pass * (D + 1)

    segs_per_core = math.ceil(N / N_CORES)
    n_quad = math.ceil(segs_per_core / QUAD)

    # sub-window seg ranges: [core][quad][sub] -> (lo, hi)
    lo_t = np.empty((N_CORES, n_quad, SPQ), dtype=np.int64)
    hi_t = np.empty((N_CORES, n_quad, SPQ), dtype=np.int64)
    for k in range(N_CORES):
        core_end = min(N, (k + 1) * segs_per_core)
        for w in range(n_quad):
            for s in range(SPQ):
                lo = min(core_end, k * segs_per_core + w * QUAD + s * SUB)
                hi = min(core_end, lo + SUB)
                lo_t[k, w, s] = lo
                hi_t[k, w, s] = hi
    e_lo = np.searchsorted(sids, lo_t.ravel(), side="left").reshape(lo_t.shape)
    e_hi = np.searchsorted(sids, hi_t.ravel(), side="left").reshape(hi_t.shape)
    n_edges = e_hi - e_lo  # [K, n_quad, SPQ]
    n_cs = np.maximum(1, (n_edges.max(axis=0) + P - 1) // P).astype(int)

    ncq = n_cs.sum(axis=1)  # chunks per quad
    vals_offs = np.concatenate([[0], np.cumsum(ncq * P * row)]).astype(int)
    locs_offs = np.concatenate([[0], np.cumsum(ncq * P)]).astype(int)

    in_maps = []
    for k in range(N_CORES):
        vals_k = np.zeros(vals_offs[-1], dtype=np_mmdt)
        locs_k = np.full(locs_offs[-1], -1.0, dtype=np.float32)
        for w in range(n_quad):
            n_c_q = int(ncq[w])
            blk = np.zeros((n_c_q * P, n_pass, D + 1), dtype=np.float32)
            blk[:, 0, D] = 1.0
            lblk = np.full(n_c_q * P, -1.0, dtype=np.float32)
            pos = 0
            for s in range(SPQ):
                lo, hi = int(e_lo[k, w, s]), int(e_hi[k, w, s])
                ne = hi - lo
                slots = int(n_cs[w, s]) * P
                if ne > 0:
                    v = vals_f[lo:hi]
                    if mode == "split":
                        hi16 = v.astype(ml_dtypes.bfloat16)
                        blk[pos : pos + ne, 0, :D] = hi16
                        blk[pos : pos + ne, 1, :D] = v - hi16.astype(np.float32)
                    else:
                        blk[pos : pos + ne, 0, :D] = v
                    lblk[pos : pos + ne] = (sids[lo:hi] - lo_t[k, w, s]).astype(
                        np.float32
                    )
                pos += slots
            vals_k[vals_offs[w] : vals_offs[w + 1]] = (
                blk.reshape(n_c_q, P, row)
                .transpose(1, 0, 2)
                .astype(np_mmdt)
                .reshape(-1)
            )
            locs_k[locs_offs[w] : locs_offs[w + 1]] = (
                lblk.reshape(n_c_q, P).transpose(1, 0).reshape(-1)
            )
        in_maps.append({"vals": vals_k, "locs": locs_k})
    return (
        in_maps,
        [list(q) for q in n_cs],
        list(vals_offs),
        list(locs_offs),
        segs_per_core,
        N,
    )


def kernel(values, segment_ids, num_segments):
    mode = MODE
    in_maps, n_cs, vals_offs, locs_offs, segs_per_core, N = _prepare_inputs(
        values, segment_ids, num_segments, mode
    )
    nc = _build_program(mode, n_cs, vals_offs, locs_offs)
    trace = bool(int(os.environ.get("KTRACE", "0")))
    res = run_bass_kernel_spmd(
        nc,
        in_maps,
        list(range(N_CORES)),
        trace=trace,
        tmpdir=os.environ.get("KTRACE_DIR") or None,
    )
    global LAST_RESULT
    LAST_RESULT = res
    parts = []
    for k in range(N_CORES):
        take = min(segs_per_core, N - k * segs_per_core)
        if take > 0:
            parts.append(res.results[k]["outp"][:take])
    return np.concatenate(parts, axis=0).astype(np.float32)


# revision 12
# speedup vs baseline: 2.0101x; 1.2983x over previous
"""Segment-mean (MeanAggregator) Trainium2 kernel.

Sorted segment ids -> shard *segments* evenly across the 8 cores (host
snaps edge ranges to segment boundaries) so each core owns a disjoint row
range of the output: no cross-core reduction.

Per core, segments are processed in quads of 4 sub-windows x 32 segments
(= 128 output rows) sharing one [128, 65] fp32 PSUM accumulator: sub s
owns partitions [32s, 32s+32) via matmul tile_position=(0, 32s) col-tiling.
Edges stream in chunks of 128 (the contraction dim).  Per quad, ONE
vector-engine tensor_tensor (stride-0 broadcast APs) builds the narrow
one-hot block O[edge, chunk, j] = (loc[edge,chunk] == j), j in [0,32), and
each chunk runs TensorE matmul(s) ps[32s:32s+32] += O_c.T @ [vals | ones].
Padding slots carry loc = -1 whose one-hot row is all zero.

MODE: "split" = values as bf16 hi + bf16 lo into the same fp32 PSUM
(full-rate PE, ~2e-6 rel err); "f16" = fp16 values (~2e-4); "f32" exact
but PE runs 2 half-rate passes per matmul.

Chunk counts per (quad, sub) are maxed over cores, so the SPMD program is
identical on all 8 cores while per-core data differs.
"""

import math
import os

import ml_dtypes
import numpy as np

import concourse.bacc as bacc
import concourse.mybir as mybir
import concourse.tile as tile
from concourse.bass_utils import run_bass_kernel_spmd

N_CORES = 8
P = 128   # edges per chunk == contraction dim
SUB = 32  # segments per sub-window (one-hot width, PSUM partition slice)
SPQ = 4   # sub-windows per quad
QUAD = SUB * SPQ  # output rows per quad
D = 64

MODE = os.environ.get("KMODE", "split")

_PROGRAM_CACHE = {}


def _build_program(mode, n_cs, vals_offs, locs_offs):
    """n_cs: [n_quad][SPQ] chunk counts; *_offs: flat offsets per quad."""
    key = (mode, tuple(tuple(q) for q in n_cs))
    if key in _PROGRAM_CACHE:
        return _PROGRAM_CACHE[key]
    f32 = mybir.dt.float32
    mmdt = {
        "split": mybir.dt.bfloat16,
        "f16": mybir.dt.float16,
        "f32": f32,
    }[mode]
    n_pass = 2 if mode == "split" else 1
    n_quad = len(n_cs)
    row = n_pass * (D + 1)
    nc = bacc.Bacc(
        "TRN2", target_bir_lowering=False, debug=False, num_devices=N_CORES
    )
    vals = nc.dram_tensor("vals", [vals_offs[-1]], mmdt, kind="ExternalInput")
    locs = nc.dram_tensor("locs", [locs_offs[-1]], f32, kind="ExternalInput")
    outp = nc.dram_tensor("outp", [n_quad * QUAD, D], f32, kind="ExternalOutput")

    with tile.TileContext(nc) as tc:
        with (
            tc.tile_pool(name="const", bufs=1) as cpool,
            tc.tile_pool(name="vals", bufs=3) as vpool,
            tc.tile_pool(name="locs", bufs=3) as lpool,
            tc.tile_pool(name="oh", bufs=3) as opool,
            tc.tile_pool(name="acc", bufs=2, space="PSUM") as ppool,
            tc.tile_pool(name="res", bufs=3) as rpool,
        ):
            iota = cpool.tile([P, SUB], f32)
            nc.gpsimd.iota(
                iota[:],
                pattern=[[1, SUB]],
                base=0,
                channel_multiplier=0,
                allow_small_or_imprecise_dtypes=True,
            )
            max_ncq = max(sum(q) for q in n_cs)
            for w in range(n_quad):
                n_c_q = sum(n_cs[w])
                # alternate the two HWDGE rings (SP=sync, ACT=scalar) so big
                # input DMAs overlap instead of serializing on one FIFO ring
                dma_a = nc.sync if w % 2 == 0 else nc.scalar
                dma_b = nc.scalar if w % 2 == 0 else nc.sync
                vt = vpool.tile([P, max_ncq * row], mmdt, tag="vt")
                dma_a.dma_start(
                    out=vt[:, : n_c_q * row],
                    in_=vals[vals_offs[w] : vals_offs[w + 1]].rearrange(
                        "(p f) -> p f", p=P
                    ),
                )
                vt4 = vt[:].rearrange("p (c s e) -> p c s e", s=n_pass, e=D + 1)
                lt = lpool.tile([P, max_ncq], f32, tag="lt")
                dma_b.dma_start(
                    out=lt[:, :n_c_q],
                    in_=locs[locs_offs[w] : locs_offs[w + 1]].rearrange(
                        "(p f) -> p f", p=P
                    ),
                )
                oh = opool.tile([P, max_ncq * SUB], mmdt, tag="oh")
                oh3 = oh[:].rearrange("p (c j) -> p c j", j=SUB)
                nc.vector.tensor_tensor(
                    out=oh3[:, :n_c_q, :],
                    in0=lt[:, :n_c_q, None].to_broadcast([P, n_c_q, SUB]),
                    in1=iota[:, None, :].to_broadcast([P, n_c_q, SUB]),
                    op=mybir.AluOpType.is_equal,
                )
                ps = ppool.tile([QUAD, D + 1], f32)
                cg = 0
                for sub in range(SPQ):
                    ncs_ = n_cs[w][sub]
                    for c in range(ncs_):
                        for s in range(n_pass):
                            nc.tensor.matmul(
                                out=ps[SUB * sub : SUB * (sub + 1), :],
                                lhsT=oh3[:, cg, :],
                                rhs=vt4[:, cg, s, :],
                                start=(c == 0 and s == 0),
                                stop=(c == ncs_ - 1 and s == n_pass - 1),
                                tile_position=(0, SUB * sub),
                            )
                        cg += 1
                cnt = rpool.tile([QUAD, 1], f32, tag="cnt")
                nc.vector.tensor_scalar(
                    out=cnt[:],
                    in0=ps[:, D : D + 1],
                    scalar1=1.0,
                    scalar2=None,
                    op0=mybir.AluOpType.max,
                )
                rec = rpool.tile([QUAD, 1], f32, tag="rec")
                nc.vector.reciprocal(out=rec[:], in_=cnt[:])
                ot = rpool.tile([QUAD, D], f32, tag="ot")
                nc.scalar.mul(out=ot[:], in_=ps[:, 0:D], mul=rec[:, 0:1])
                dma_b.dma_start(
                    out=outp[w * QUAD : (w + 1) * QUAD, :], in_=ot[:]
                )
    nc.compile()
    _PROGRAM_CACHE[key] = nc
    return nc


def _prepare_inputs(values, segment_ids, num_segments, mode=MODE):
    E, d = values.shape
    assert d == D
    N = int(num_segments)
    sids = np.ascontiguousarray(np.asarray(segment_ids, dtype=np.int64))
    vals_f = np.ascontiguousarray(np.asarray(values), dtype=np.float32)

    n_pass = 2 if mode == "split" else 1
    np_mmdt = {
        "split": ml_dtypes.bfloat16,
        "f16": np.float16,
        "f32": np.float32,
    }[mode]
    row = n_pass * (D + 1)

    segs_per_core = math.ceil(N / N_CORES)
    n_quad = math.ceil(segs_per_core / QUAD)

    # sub-window seg ranges: [core][quad][sub] -> (lo, hi)
    lo_t = np.empty((N_CORES, n_quad, SPQ), dtype=np.int64)
    hi_t = np.empty((N_CORES, n_quad, SPQ), dtype=np.int64)
    for k in range(N_CORES):
        core_end = min(N, (k + 1) * segs_per_core)
        for w in range(n_quad):
            for s in range(SPQ):
                lo = min(core_end, k * segs_per_core + w * QUAD + s * SUB)
                hi = min(core_end, lo + SUB)
                lo_t[k, w, s] = lo
                hi_t[k, w, s] = hi
    e_lo = np.searchsorted(sids, lo_t.ravel(), side="left").reshape(lo_t.shape)
    e_hi = np.searchsorted(sids, hi_t.ravel(), side="left").reshape(hi_t.shape)
    n_edges = e_hi - e_lo  # [K, n_quad, SPQ]
    n_cs = np.maximum(1, (n_edges.max(axis=0) + P - 1) // P).astype(int)

    ncq = n_cs.sum(axis=1)  # chunks per quad
    vals_offs = np.concatenate([[0], np.cumsum(ncq * P * row)]).astype(int)
    locs_offs = np.concatenate([[0], np.cumsum(ncq * P)]).astype(int)

    in_maps = []
    for k in range(N_CORES):
        vals_k = np.zeros(vals_offs[-1], dtype=np_mmdt)
        locs_k = np.full(locs_offs[-1], -1.0, dtype=np.float32)
        for w in range(n_quad):
            n_c_q = int(ncq[w])
            blk = np.zeros((n_c_q * P, n_pass, D + 1), dtype=np.float32)
            blk[:, 0, D] = 1.0
            lblk = np.full(n_c_q * P, -1.0, dtype=np.float32)
            pos = 0
            for s in range(SPQ):
                lo, hi = int(e_lo[k, w, s]), int(e_hi[k, w, s])
                ne = hi - lo
                slots = int(n_cs[w, s]) * P
                if ne > 0:
                    v = vals_f[lo:hi]
                    if mode == "split":
                        hi16 = v.astype(ml_dtypes.bfloat16)
                        blk[pos : pos + ne, 0, :D] = hi16
                        blk[pos : pos + ne, 1, :D] = v - hi16.astype(np.float32)
                    else:
                        blk[pos : pos + ne, 0, :D] = v
                    lblk[pos : pos + ne] = (sids[lo:hi] - lo_t[k, w, s]).astype(
                        np.float32
                    )
                pos += slots
            vals_k[vals_offs[w] : vals_offs[w + 1]] = (
                blk.reshape(n_c_q, P, row)
                .transpose(1, 0, 2)
                .astype(np_mmdt)
                .reshape(-1)
            )
            locs_k[locs_offs[w] : locs_offs[w + 1]] = (
                lblk.reshape(n_c_q, P).transpose(1, 0).reshape(-1)
            )
        in_maps.append({"vals": vals_k, "locs": locs_k})
    return (
        in_maps,
        [list(q) for q in n_cs],
        list(vals_offs),
        list(locs_offs),
        segs_per_core,
        N,
    )


def kernel(values, segment_ids, num_segments):
    mode = MODE
    in_maps, n_cs, vals_offs, locs_offs, segs_per_core, N = _prepare_inputs(
        values, segment_ids, num_segments, mode
    )
    nc = _build_program(mode, n_cs, vals_offs, locs_offs)
    trace = bool(int(os.environ.get("KTRACE", "0")))
    res = run_bass_kernel_spmd(
        nc,
        in_maps,
        list(range(N_CORES)),
        trace=trace,
        tmpdir=os.environ.get("KTRACE_DIR") or None,
    )
    global LAST_RESULT
    LAST_RESULT = res
    parts = []
    for k in range(N_CORES):
        take = min(segs_per_core, N - k * segs_per_core)
        if take > 0:
            parts.append(res.results[k]["outp"][:take])
    return np.concatenate(parts, axis=0).astype(np.float32)
